# revision 1
# baseline (speedup 1.0000x reference)
"""Trainium2 Bass kernel for HSEGNNFlexLayer (GNN message passing).

Strategy (8 NeuronCores, SPMD, zero collectives):
  - Host assigns each node to a (core, window, slot) bin: 8 cores x 25
    windows x 256 slots.  Every edge is routed to the core that owns its
    dst node, so the segment-sum is fully local to each core.
  - Per core, edges are grouped by window and padded to a uniform tile
    grid (NWIN x T_B x 128) so one Bass program serves all 8 cores.
  - Message layers: c = a @ Wflat computed with edges on PSUM partitions
    (lhsT = transposed, host-gathered features), attr-weighted k-sum via
    per-partition scalar_tensor_tensor chains, Silu on ScalarE.
  - Scatter-add: one-hot S matmul (lhsT=m2, rhs=S) accumulating into a
    per-window PSUM bank; flushed to an SBUF-resident transposed
    aggregate.
  - Node update layers run the same pipeline over the 6400 node slots.
"""

import numpy as np
import ml_dtypes

import concourse.bass as bass
import concourse.mybir as mybir
import concourse.tile as tile
from concourse import bacc
from concourse import bass_utils
from concourse.masks import make_identity

# Problem constants (hardcoded per contest contract)
N, E, D, A, AM = 50000, 500000, 128, 8, 3
MIN_DIM = 2 * D + AM  # 259
UIN_DIM = D + D + AM  # 259
NCORES = 8
P = 128
KO = A * D  # 1024 = flattened (k, o) output columns per TP layer
SLOTS = 256  # node slots per window (one PSUM bank of f32)
NWIN = 25
NODE_SLOTS = NWIN * SLOTS  # 6400 per core
BF16 = mybir.dt.bfloat16
F32 = mybir.dt.float32
NPBF16 = ml_dtypes.bfloat16

_cache = {}


# --------------------------------------------------------------------------
# Host-side preparation
# --------------------------------------------------------------------------

def _assign_nodes(dst):
    """Greedy-pack nodes into NCORES*NWIN bins (<=SLOTS nodes each),
    balancing per-bin edge counts.  Returns (node2bin, node2slot)."""
    import heapq

    counts = np.bincount(dst, minlength=N)
    order = np.argsort(-counts, kind="stable")
    nbins = NCORES * NWIN
    node2bin = np.empty(N, dtype=np.int32)
    node2slot = np.empty(N, dtype=np.int32)
    bin_nodes = np.zeros(nbins, dtype=np.int32)
    # heap of (edge_count, bin)
    heap = [(0, b) for b in range(nbins)]
    heapq.heapify(heap)
    pending = []
    for n in order:
        while True:
            c, b = heapq.heappop(heap)
            if bin_nodes[b] < SLOTS:
                break
            pending.append((c, b))  # full bin: drop permanently
        node2bin[n] = b
        node2slot[n] = bin_nodes[b]
        bin_nodes[b] += 1
        heapq.heappush(heap, (c + int(counts[n]), b))
    return node2bin, node2slot


def _prepare(x, edge_attr, node_attr, amf, anf, W1, b1, W2, b2, W3, b3, W4, b4,
             edge_index):
    x = np.asarray(x, dtype=np.float32)
    edge_attr = np.asarray(edge_attr, dtype=np.float32)
    node_attr = np.asarray(node_attr, dtype=np.float32)
    amf = np.asarray(amf, dtype=np.float32)
    anf = np.asarray(anf, dtype=np.float32)
    src = np.asarray(edge_index[0], dtype=np.int64).astype(np.int32)
    dst = np.asarray(edge_index[1], dtype=np.int64).astype(np.int32)

    node2bin, node2slot = _assign_nodes(dst)
    node_core = node2bin // NWIN
    node_win = node2bin % NWIN
    node_gslot = node_win * SLOTS + node2slot  # slot within core [0, NODE_SLOTS)

    e_bin = node2bin[dst]  # bin (core*NWIN + win) of each edge

    # order edges by bin
    e_order = np.argsort(e_bin, kind="stable")
    e_bin_sorted = e_bin[e_order]
    bin_cnt = np.bincount(e_bin_sorted, minlength=NCORES * NWIN)
    # tiles per window: uniform across all bins
    T_B = int(np.ceil(bin_cnt.max() / P))
    win_cap = T_B * P
    E_pad = NWIN * win_cap

    bin_starts = np.zeros(NCORES * NWIN + 1, dtype=np.int64)
    np.cumsum(bin_cnt, out=bin_starts[1:])

    # Destination position of each (sorted) edge inside its core's padded list
    offs_in_bin = np.arange(len(e_order)) - bin_starts[e_bin_sorted]
    pos = (e_bin_sorted % NWIN) * win_cap + offs_in_bin  # position within core
    core_of_edge = e_bin_sorted // NWIN

    # Per-core packed index arrays (padded entries use sentinel -1)
    ew_src = np.full((NCORES, E_pad), -1, dtype=np.int64)
    ew_dst = np.full((NCORES, E_pad), -1, dtype=np.int64)
    ew_eid = np.full((NCORES, E_pad), -1, dtype=np.int64)
    ew_src[core_of_edge, pos] = src[e_order]
    ew_dst[core_of_edge, pos] = dst[e_order]
    ew_eid[core_of_edge, pos] = e_order

    # Flattened weights (k-major columns): Wf[i, k*D + o] = W[i, k, o]
    w1f = np.ascontiguousarray(np.asarray(W1, np.float32).reshape(MIN_DIM, KO)).astype(NPBF16)
    w2f = np.ascontiguousarray(np.asarray(W2, np.float32).reshape(D, KO)).astype(NPBF16)
    w3f = np.ascontiguousarray(np.asarray(W3, np.float32).reshape(UIN_DIM, KO)).astype(NPBF16)
    w4f = np.ascontiguousarray(np.asarray(W4, np.float32).reshape(D, KO)).astype(NPBF16)
    biases = [np.ascontiguousarray(np.tile(np.asarray(b, np.float32)[None, :], (P, 1)))
              for b in (b1, b2, b3, b4)]

    xT_all = x.T.astype(NPBF16)  # [D, N]

    in_maps = []
    slot2node = np.full((NCORES, NODE_SLOTS), -1, dtype=np.int64)
    for c in range(NCORES):
        s = ew_src[c]
        d = ew_dst[c]
        eid = ew_eid[c]
        valid = eid >= 0
        sv = np.where(valid, s, 0)
        dv = np.where(valid, d, 0)
        ev = np.where(valid, eid, 0)

        xiT = xT_all[:, dv].copy()
        xjT = xT_all[:, sv].copy()
        xiT[:, ~valid] = 0
        xjT[:, ~valid] = 0
        amfT = amf[ev].T.astype(NPBF16)
        amfT[:, ~valid] = 0
        battr = edge_attr[ev].astype(np.float32)
        battr[~valid] = 0

        # scatter one-hot: local slot within window
        S = np.zeros((E_pad, SLOTS), dtype=NPBF16)
        rows = np.nonzero(valid)[0]
        S[rows, node2slot[d[rows]]] = 1

        # node side
        nodes_c = np.nonzero(node_core == c)[0]
        gs = node_gslot[nodes_c]
        slot2node[c, gs] = nodes_c
        nxT = np.zeros((D, NODE_SLOTS), dtype=NPBF16)
        nxT[:, gs] = xT_all[:, nodes_c]
        nanfT = np.zeros((AM, NODE_SLOTS), dtype=NPBF16)
        nanfT[:, gs] = anf[nodes_c].T.astype(NPBF16)
        nattr = np.zeros((NODE_SLOTS, A), dtype=np.float32)
        nattr[gs] = node_attr[nodes_c]

        in_maps.append({
            "xiT": np.ascontiguousarray(xiT),
            "xjT": np.ascontiguousarray(xjT),
            "amfT": np.ascontiguousarray(amfT),
            "battr": np.ascontiguousarray(battr),
            "S": S,
            "xT": nxT,
            "anfT": nanfT,
            "nattr": nattr,
            "w1f": w1f, "w2f": w2f, "w3f": w3f, "w4f": w4f,
            "b1r": biases[0], "b2r": biases[1], "b3r": biases[2], "b4r": biases[3],
        })
    return in_maps, slot2node, T_B, E_pad


# --------------------------------------------------------------------------
# Device kernel builder
# --------------------------------------------------------------------------

def _build(T_B, E_pad):
    nc = bacc.Bacc("TRN2", target_bir_lowering=False, debug=False,
                   num_devices=NCORES)

    d_xiT = nc.dram_tensor("xiT", [D, E_pad], BF16, kind="ExternalInput")
    d_xjT = nc.dram_tensor("xjT", [D, E_pad], BF16, kind="ExternalInput")
    d_amfT = nc.dram_tensor("amfT", [AM, E_pad], BF16, kind="ExternalInput")
    d_battr = nc.dram_tensor("battr", [E_pad, A], F32, kind="ExternalInput")
    d_S = nc.dram_tensor("S", [E_pad, SLOTS], BF16, kind="ExternalInput")
    d_xT = nc.dram_tensor("xT", [D, NODE_SLOTS], BF16, kind="ExternalInput")
    d_anfT = nc.dram_tensor("anfT", [AM, NODE_SLOTS], BF16, kind="ExternalInput")
    d_nattr = nc.dram_tensor("nattr", [NODE_SLOTS, A], F32, kind="ExternalInput")
    d_w1f = nc.dram_tensor("w1f", [MIN_DIM, KO], BF16, kind="ExternalInput")
    d_w2f = nc.dram_tensor("w2f", [D, KO], BF16, kind="ExternalInput")
    d_w3f = nc.dram_tensor("w3f", [UIN_DIM, KO], BF16, kind="ExternalInput")
    d_w4f = nc.dram_tensor("w4f", [D, KO], BF16, kind="ExternalInput")
    d_b = [nc.dram_tensor(f"b{i}r", [P, D], F32, kind="ExternalInput")
           for i in (1, 2, 3, 4)]
    d_out = nc.dram_tensor("out", [NODE_SLOTS, D], F32, kind="ExternalOutput")

    mult = mybir.AluOpType.mult
    add = mybir.AluOpType.add
    silu = mybir.ActivationFunctionType.Silu

    with tile.TileContext(nc) as tc:
        with (
            tc.tile_pool(name="const", bufs=1) as cpool,
            tc.tile_pool(name="ain", bufs=3) as apool,
            tc.tile_pool(name="work", bufs=3) as wpool,
            tc.tile_pool(name="cps", bufs=2, space="PSUM") as cps,
            tc.tile_pool(name="trps", bufs=2, space="PSUM") as trps,
            tc.tile_pool(name="aggps", bufs=1, space="PSUM") as aggps,
        ):
            # ---- constants resident in SBUF ----
            ident = cpool.tile([P, P], BF16, tag="ident", name="ident")
            make_identity(nc, ident[:])

            w1c = [cpool.tile([P, KO], BF16, tag="w1c0", name="w1c0"),
                   cpool.tile([P, KO], BF16, tag="w1c1", name="w1c1"),
                   cpool.tile([AM, KO], BF16, tag="w1c2", name="w1c2")]
            nc.sync.dma_start(w1c[0][:], d_w1f.ap()[0:P, :])
            nc.sync.dma_start(w1c[1][:], d_w1f.ap()[P:2 * P, :])
            nc.sync.dma_start(w1c[2][:], d_w1f.ap()[2 * P:MIN_DIM, :])
            w2c = cpool.tile([P, KO], BF16, tag="w2c", name="w2c")
            nc.sync.dma_start(w2c[:], d_w2f.ap())
            w3c = [cpool.tile([P, KO], BF16, tag="w3c0", name="w3c0"),
                   cpool.tile([P, KO], BF16, tag="w3c1", name="w3c1"),
                   cpool.tile([AM, KO], BF16, tag="w3c2", name="w3c2")]
            nc.sync.dma_start(w3c[0][:], d_w3f.ap()[0:P, :])
            nc.sync.dma_start(w3c[1][:], d_w3f.ap()[P:2 * P, :])
            nc.sync.dma_start(w3c[2][:], d_w3f.ap()[2 * P:UIN_DIM, :])
            w4c = cpool.tile([P, KO], BF16, tag="w4c", name="w4c")
            nc.sync.dma_start(w4c[:], d_w4f.ap())

            btile = [cpool.tile([P, D], F32, tag=f"b{i}r", name=f"b{i}r")
                     for i in range(4)]
            for i in range(4):
                nc.sync.dma_start(btile[i][:], d_b[i].ap())

            aggT = cpool.tile([P, NODE_SLOTS], BF16, tag="aggT", name="aggT")

            # ---- helper: one TP layer tile (c = lhs-chunks @ wflat,
            #      weighted k-sum + bias, optional silu) ----
            def tp_layer(chunks, wchunks, bt, bias_rep, out_tile, do_silu):
                cpsum = cps.tile([P, KO], F32, tag="c", name="c")
                nch = len(chunks)
                for ci in range(nch):
                    for h in range(2):
                        nc.tensor.matmul(
                            cpsum[:, h * 512:(h + 1) * 512],
                            lhsT=chunks[ci],
                            rhs=wchunks[ci][:, h * 512:(h + 1) * 512],
                            start=(ci == 0),
                            stop=(ci == nch - 1),
                        )
                acc = wpool.tile([P, D], F32, tag="acc", name="acc")
                nc.vector.scalar_tensor_tensor(
                    acc[:], cpsum[:, 0:D], bt[:, 0:1], bias_rep[:], mult, add)
                for k in range(1, A):
                    nc.vector.scalar_tensor_tensor(
                        acc[:], cpsum[:, k * D:(k + 1) * D], bt[:, k:k + 1],
                        acc[:], mult, add)
                if do_silu:
                    nc.scalar.activation(out_tile[:], acc[:], silu)
                else:
                    nc.vector.tensor_copy(out_tile[:], acc[:])

            def transpose_to(src_bf16):
                tps = trps.tile([P, P], BF16, tag="tr", name="tr")
                nc.tensor.transpose(tps[:], src_bf16[:], ident[:])
                dst = wpool.tile([P, P], BF16, tag="mT", name="mT")
                nc.vector.tensor_copy(dst[:], tps[:])
                return dst

            # ---- edge phase ----
            GT = 4  # tiles fetched per DMA group
            agg_hold = [None]
            ntiles = NWIN * T_B
            for g0 in range(0, ntiles, GT):
                gn = min(GT, ntiles - g0)
                e0 = g0 * P
                ew = gn * P
                xi4 = apool.tile([P, GT * P], BF16, tag="xi4", name="xi4")
                xj4 = apool.tile([P, GT * P], BF16, tag="xj4", name="xj4")
                am4 = apool.tile([AM, GT * P], BF16, tag="am4", name="am4")
                nc.sync.dma_start(xi4[:, :ew], d_xiT.ap()[:, e0:e0 + ew])
                nc.sync.dma_start(xj4[:, :ew], d_xjT.ap()[:, e0:e0 + ew])
                nc.sync.dma_start(am4[:, :ew], d_amfT.ap()[:, e0:e0 + ew])
                for j in range(gn):
                    t = g0 + j
                    w = t // T_B
                    tw = t % T_B
                    bt = apool.tile([P, A], F32, tag="bt", name="bt")
                    nc.sync.dma_start(
                        bt[:], d_battr.ap()[t * P:(t + 1) * P, :])
                    St = apool.tile([P, SLOTS], BF16, tag="St", name="St")
                    nc.sync.dma_start(
                        St[:], d_S.ap()[t * P:(t + 1) * P, :])

                    m1 = wpool.tile([P, D], BF16, tag="m1", name="m1")
                    tp_layer([xi4[:, j * P:(j + 1) * P],
                              xj4[:, j * P:(j + 1) * P],
                              am4[:, j * P:(j + 1) * P]],
                             w1c, bt, btile[0], m1, True)
                    m1T = transpose_to(m1)
                    m2 = wpool.tile([P, D], BF16, tag="m2", name="m2")
                    tp_layer([m1T], [w2c], bt, btile[1], m2, True)

                    if tw == 0:
                        agg_hold[0] = aggps.tile([P, SLOTS], F32, tag="agg", name="agg")
                    agg_ps = agg_hold[0]
                    nc.tensor.matmul(
                        agg_ps[:],
                        lhsT=m2[:],
                        rhs=St[:],
                        start=(tw == 0),
                        stop=(tw == T_B - 1),
                    )
                    if tw == T_B - 1:
                        nc.vector.tensor_copy(
                            aggT[:, w * SLOTS:(w + 1) * SLOTS], agg_ps[:])

            # ---- node phase ----
            nnt = NODE_SLOTS // P  # 50
            for g0 in range(0, nnt, GT):
                gn = min(GT, nnt - g0)
                n0 = g0 * P
                nw = gn * P
                xt4 = apool.tile([P, GT * P], BF16, tag="xi4", name="xi4")
                an4 = apool.tile([AM, GT * P], BF16, tag="am4", name="am4")
                nc.sync.dma_start(xt4[:, :nw], d_xT.ap()[:, n0:n0 + nw])
                nc.sync.dma_start(an4[:, :nw], d_anfT.ap()[:, n0:n0 + nw])
                for j in range(gn):
                    t = g0 + j
                    na = apool.tile([P, A], F32, tag="bt", name="bt")
                    nc.sync.dma_start(
                        na[:], d_nattr.ap()[t * P:(t + 1) * P, :])
                    u = wpool.tile([P, D], BF16, tag="m1", name="m1")
                    tp_layer([xt4[:, j * P:(j + 1) * P],
                              aggT[:, t * P:(t + 1) * P],
                              an4[:, j * P:(j + 1) * P]],
                             w3c, na, btile[2], u, True)
                    uT = transpose_to(u)
                    out_t = wpool.tile([P, D], F32, tag="outt", name="outt")
                    tp_layer([uT], [w4c], na, btile[3], out_t, False)
                    nc.sync.dma_start(
                        d_out.ap()[t * P:(t + 1) * P, :], out_t[:])

    nc.compile()
    return nc


# --------------------------------------------------------------------------
# Entry point
# --------------------------------------------------------------------------

def kernel(x, edge_attr, node_attr, additional_message_features,
           additional_node_features, W1, b1, W2, b2, W3, b3, W4, b4,
           edge_index, batch=None):
    in_maps, slot2node, T_B, E_pad = _prepare(
        x, edge_attr, node_attr, additional_message_features,
        additional_node_features, W1, b1, W2, b2, W3, b3, W4, b4, edge_index)

    key = (T_B, E_pad)
    if key not in _cache:
        _cache[key] = _build(T_B, E_pad)
    nc = _cache[key]

    res = bass_utils.run_bass_kernel_spmd(
        nc, in_maps, core_ids=list(range(NCORES)))
    kernel.last = (nc, in_maps, res)

    out = np.zeros((N, D), dtype=np.float32)
    for c in range(NCORES):
        oc = res.results[c]["out"]
        mask = slot2node[c] >= 0
        out[slot2node[c][mask]] = oc[mask]
    return out

